# revision 10
# baseline (speedup 1.0000x reference)
"""GCN message-passing layer on 8 Trainium2 NeuronCores.

out = segment_sum(x[src], dst, N) @ W.T + b

Strategy (per core, dst-sharded):
  - Host: greedily bin-pack dst nodes into 320 bins (<=128 nodes each),
    balancing in-degree; bins are dealt to (core, tile) slots sorted by
    A-half edge count so every core's tile t has a near-identical edge
    count (the SPMD program is shared across cores, so per-tile stream
    lengths are the max over cores -- aligning counts keeps padding tiny).
    Edges form two unpadded slot streams (A: src<32768, B: rest -- int16
    gather indices force splitting the x table), 128-slot chunks crossing
    tile boundaries. x is pre-cast to bf16 on host.
  - Device: the whole bf16 message stream lives in two SBUF arenas
    (~160 KB/partition); dma_gather (SWDGE) calls of 8 chunks (1024
    indices -- the SWDGE ring limit) stream x rows in, A/B interleaved by
    first-tile. Each (chunk, tile) piece gets a one-hot matrix P (DVE
    iota==dst_lane, bf16; out-of-tile slots hit the PAD sentinel and give
    zero columns) and a PE matmul (bf16, 1 cyc/row) accumulating into the
    tile's PSUM; a second PE matmul applies W.T and the bias.
  - Host: inverse-permute the per-core outputs back to node order.
"""

import sys

import numpy as np

sys.path.insert(0, "/opt/trn_rl_repo")

N_NODES = 40000
N_EDGES = 640000
D = 128
P = 128
N_CORES = 8
TILES_PER_CORE = 40
N_BINS = N_CORES * TILES_PER_CORE  # 320 tiles of up to 128 nodes
SLOTS_PER_CORE = TILES_PER_CORE * P  # 5120
HALF = 32768  # int16 gather index limit
PAD_DST = 1000.0  # dst_local sentinel for pad slots -> all-zero one-hot column
CALL_CHUNKS = 8  # 1024 indices per dma_gather call -- hard SWDGE ring limit

_PROGRAM_CACHE: dict = {}


def _pack_nodes(dst: np.ndarray):
    """Greedy LPT pack of nodes into N_BINS bins (<=128 nodes each),
    balancing per-bin in-degree. Returns bin/lane maps."""
    import heapq

    deg = np.bincount(dst, minlength=N_NODES)
    order = np.argsort(-deg, kind="stable")

    heap = [(0, b) for b in range(N_BINS)]
    heapq.heapify(heap)
    counts = np.zeros(N_BINS, dtype=np.int64)
    node_bin = np.empty(N_NODES, dtype=np.int32)
    node_lane = np.empty(N_NODES, dtype=np.int32)
    for n in order:
        while True:
            s, b = heapq.heappop(heap)
            if counts[b] < P:
                break
        node_bin[n] = b
        node_lane[n] = counts[b]
        counts[b] += 1
        heapq.heappush(heap, (s + int(deg[n]), b))
    return node_bin, node_lane


def _wrap_idx(arr: np.ndarray) -> np.ndarray:
    """int16 flat idx list -> [128, len/16] wrapped + replicated layout."""
    w = arr.reshape(-1, 16).T  # [16, n/16]
    return np.ascontiguousarray(np.tile(w, (8, 1)))


def _piece_layout(maxA, maxB):
    """Shared host/device layout: unpadded slot streams, 128-slot chunks,
    and per-tile (stream, chunk) pieces with their dl column indices."""
    maxA = np.asarray(maxA)
    maxB = np.asarray(maxB)
    SA = np.concatenate(([0], np.cumsum(maxA)))  # slot offsets, A stream
    SB = np.concatenate(([0], np.cumsum(maxB)))
    KA = -(-int(SA[-1]) // P)  # chunks (incl. padded tail chunk)
    KB = -(-int(SB[-1]) // P)
    pieces = []  # per tile: list of (stream, chunk)
    col = 0
    col0 = []
    for t in range(TILES_PER_CORE):
        pl = []
        for stream, S in ((0, SA), (1, SB)):
            k0 = int(S[t]) // P
            k1 = (int(S[t + 1]) - 1) // P
            for k in range(k0, k1 + 1):
                pl.append((stream, k))
        col0.append(col)
        col += len(pl)
        pieces.append(pl)
    return SA, SB, KA, KB, pieces, col0, col


def _prepare(x, src, dst, W, b):
    import ml_dtypes

    src = np.asarray(src).astype(np.int64)
    dst = np.asarray(dst).astype(np.int64)
    node_bin, node_lane = _pack_nodes(dst)

    # Deal bins (sorted by A-half count) to (core, tile) slots: sorted rank
    # i -> core i%8, tile i//8. Aligns per-tile counts across cores and makes
    # tile sizes descending within each core.
    ehalf = (src >= HALF).astype(np.int64)
    binA = np.bincount(node_bin[dst[ehalf == 0]], minlength=N_BINS)
    rank = np.argsort(-binA, kind="stable")
    bin_core = np.empty(N_BINS, dtype=np.int64)
    bin_tile = np.empty(N_BINS, dtype=np.int64)
    bin_core[rank] = np.arange(N_BINS) % N_CORES
    bin_tile[rank] = np.arange(N_BINS) // N_CORES

    ebin = node_bin[dst]
    etile = bin_core[ebin] * TILES_PER_CORE + bin_tile[ebin]
    edl = node_lane[dst].astype(np.float32)
    eorder = np.lexsort((src, ehalf, etile))
    s_src, s_dl = src[eorder], edl[eorder]
    key = etile[eorder] * 2 + ehalf[eorder]

    seg_start = np.searchsorted(key, np.arange(N_BINS * 2), side="left")
    seg_end = np.searchsorted(key, np.arange(N_BINS * 2), side="right")
    cntA = (seg_end[0::2] - seg_start[0::2]).reshape(N_CORES, TILES_PER_CORE)
    cntB = (seg_end[1::2] - seg_start[1::2]).reshape(N_CORES, TILES_PER_CORE)
    maxA = cntA.max(axis=0)
    maxB = np.maximum(1, cntB.max(axis=0))
    SA, SB, KA, KB, pieces, col0, ncol = _piece_layout(maxA, maxB)

    x_bf = np.asarray(x).astype(ml_dtypes.bfloat16)
    xa = np.ascontiguousarray(x_bf[:HALF])
    xb = np.ascontiguousarray(x_bf[HALF:])
    iota = np.tile(
        np.arange(P, dtype=np.float32)[None, :], (P, 1)
    ).astype(ml_dtypes.bfloat16)
    wt = np.ascontiguousarray(np.asarray(W).T.astype(ml_dtypes.bfloat16))
    brow = np.asarray(b).astype(ml_dtypes.bfloat16)[None, :]

    # per-tile dl column index of each stream's first piece
    colA0 = np.array(col0)
    nApieces = np.array(
        [sum(1 for st, _ in pl if st == 0) for pl in pieces]
    )
    colB0 = colA0 + nApieces
    kA0 = SA[:-1] // P  # first A chunk of each tile
    kB0 = SB[:-1] // P

    in_maps = []
    for c in range(N_CORES):
        idxA = np.zeros(KA * P, dtype=np.int16)
        idxB = np.zeros(KB * P, dtype=np.int16)
        dl = np.full((ncol, P), PAD_DST, dtype=np.float32)
        for t in range(TILES_PER_CORE):
            g = (c * TILES_PER_CORE + t) * 2
            a0, a1 = seg_start[g], seg_end[g]
            nA = a1 - a0
            slots = int(SA[t]) + np.arange(nA)
            idxA[slots] = s_src[a0:a1].astype(np.int16)
            dl[colA0[t] + slots // P - kA0[t], slots % P] = s_dl[a0:a1]
            b0, b1 = seg_start[g + 1], seg_end[g + 1]
            nB = b1 - b0
            slots = int(SB[t]) + np.arange(nB)
            idxB[slots] = (s_src[b0:b1] - HALF).astype(np.int16)
            dl[colB0[t] + slots // P - kB0[t], slots % P] = s_dl[b0:b1]

        in_maps.append(
            {
                "x": xa,
                "xb": xb,
                "idxa": _wrap_idx(idxA),
                "idxb": _wrap_idx(idxB),
                "dstloc": np.ascontiguousarray(dl.T),  # [128 lanes, ncol]
                "wt": wt,
                "iota": iota,
                "brow": brow,
            }
        )

    slot = (bin_core[node_bin] * TILES_PER_CORE + bin_tile[node_bin]) * P + node_lane
    slot_node = np.full(N_BINS * P, -1, dtype=np.int64)
    slot_node[slot] = np.arange(N_NODES)
    caps = (tuple(int(v) for v in maxA), tuple(int(v) for v in maxB))
    return in_maps, caps, slot_node


def _build_program(maxA: tuple, maxB: tuple):
    import concourse.mybir as mybir
    import concourse.tile as tile
    from concourse import bacc

    f32 = mybir.dt.float32
    bf16 = mybir.dt.bfloat16
    i16 = mybir.dt.int16

    SA, SB, KA, KB, pieces, col0, ncol = _piece_layout(maxA, maxB)

    nc = bacc.Bacc("TRN2")
    x = nc.dram_tensor("x", [HALF, D], bf16, kind="ExternalInput")
    xb = nc.dram_tensor("xb", [N_NODES - HALF, D], bf16, kind="ExternalInput")
    idxa = nc.dram_tensor("idxa", [P, KA * P // 16], i16, kind="ExternalInput")
    idxb = nc.dram_tensor("idxb", [P, KB * P // 16], i16, kind="ExternalInput")
    dstloc = nc.dram_tensor("dstloc", [P, ncol], f32, kind="ExternalInput")
    wt = nc.dram_tensor("wt", [D, D], bf16, kind="ExternalInput")
    iota_in = nc.dram_tensor("iota", [P, P], bf16, kind="ExternalInput")
    brow = nc.dram_tensor("brow", [1, D], bf16, kind="ExternalInput")
    out = nc.dram_tensor("out", [SLOTS_PER_CORE, D], f32, kind="ExternalOutput")

    # Gather calls: CALL_CHUNKS-chunk groups of each stream, interleaved by
    # the first tile each call serves so tile processing chases the stream.
    tile_of_a = np.searchsorted(SA[1:], np.arange(KA, dtype=np.int64) * P, side="right")
    tile_of_b = np.searchsorted(SB[1:], np.arange(KB, dtype=np.int64) * P, side="right")
    calls = []
    for s in range(0, KA, CALL_CHUNKS):
        e = min(s + CALL_CHUNKS, KA)
        calls.append((int(tile_of_a[s]), 0, s, e))
    for s in range(0, KB, CALL_CHUNKS):
        e = min(s + CALL_CHUNKS, KB)
        calls.append((int(tile_of_b[s]), 1, s, e))
    calls.sort()

    # idx tables load in two pieces: a small head covering the first calls
    # (so the DMA pipeline starts immediately), then the remainder.
    headA = min(4 * CALL_CHUNKS * P // 16, KA * P // 16)
    headB = min(2 * CALL_CHUNKS * P // 16, KB * P // 16)

    with tile.TileContext(nc) as tc:
        with (
            tc.tile_pool(name="const", bufs=1) as cpool,
            tc.tile_pool(name="pt", bufs=8) as p_pool,
            tc.tile_pool(name="ht", bufs=4) as ht_pool,
            tc.tile_pool(name="ot", bufs=4) as o_pool,
            tc.tile_pool(name="ps1", bufs=4, space="PSUM") as ps1_pool,
            tc.tile_pool(name="ps2", bufs=2, space="PSUM") as ps2_pool,
        ):
            idxa_t = cpool.tile([P, KA * P // 16], i16)
            nc.sync.dma_start(out=idxa_t[:, :headA], in_=idxa[:, :headA])
            idxb_t = cpool.tile([P, KB * P // 16], i16)
            nc.sync.dma_start(out=idxb_t[:, :headB], in_=idxb[:, :headB])
            if headA < KA * P // 16:
                nc.sync.dma_start(out=idxa_t[:, headA:], in_=idxa[:, headA:])
            if headB < KB * P // 16:
                nc.sync.dma_start(out=idxb_t[:, headB:], in_=idxb[:, headB:])
            dl_t = cpool.tile([P, ncol], f32)
            nc.sync.dma_start(out=dl_t[:], in_=dstloc[:])
            iota_t = cpool.tile([P, P], bf16)
            nc.sync.dma_start(out=iota_t[:], in_=iota_in[:])
            wt_t = cpool.tile([D, D], bf16)
            nc.sync.dma_start(out=wt_t[:], in_=wt[:])
            b_t = cpool.tile([1, D], bf16)
            nc.sync.dma_start(out=b_t[:], in_=brow[:])
            ones_t = cpool.tile([1, P], bf16)
            nc.vector.memset(ones_t[:], 1.0)

            # whole-stream SBUF arenas; each chunk written exactly once
            arena_a = cpool.tile([P, KA, D], bf16)
            arena_b = cpool.tile([P, KB, D], bf16)
            for _, stream, s, e in calls:
                n = (e - s) * P
                if stream == 0:
                    nc.gpsimd.dma_gather(
                        out_ap=arena_a[:, s:e, :],
                        in_ap=x[:],
                        idxs_ap=idxa_t[:, s * P // 16 : e * P // 16],
                        num_idxs=n,
                        num_idxs_reg=n,
                        elem_size=D,
                        elem_step=D,
                    )
                else:
                    nc.gpsimd.dma_gather(
                        out_ap=arena_b[:, s:e, :],
                        in_ap=xb[:],
                        idxs_ap=idxb_t[:, s * P // 16 : e * P // 16],
                        num_idxs=n,
                        num_idxs_reg=n,
                        elem_size=D,
                        elem_step=D,
                    )

            for t in range(TILES_PER_CORE):
                pl = pieces[t]
                ps_ht = ps1_pool.tile([P, P], f32, tag="psht")
                for pi, (stream, k) in enumerate(pl):
                    col = col0[t] + pi
                    pt = p_pool.tile([P, P], bf16, tag="pt")
                    nc.vector.tensor_scalar(
                        out=pt[:],
                        in0=iota_t[:],
                        scalar1=dl_t[:, col : col + 1],
                        scalar2=None,
                        op0=mybir.AluOpType.is_equal,
                    )
                    arena = arena_a if stream == 0 else arena_b
                    nc.tensor.matmul(
                        out=ps_ht[:],
                        lhsT=arena[:, k, :],
                        rhs=pt[:],
                        start=(pi == 0),
                        stop=(pi == len(pl) - 1),
                    )
                ht_t = ht_pool.tile([P, P], bf16, tag="ht")
                nc.scalar.copy(out=ht_t[:], in_=ps_ht[:])
                ps_o = ps2_pool.tile([P, D], f32, tag="pso")
                nc.tensor.matmul(
                    out=ps_o[:], lhsT=ht_t[:], rhs=wt_t[:], start=True, stop=False
                )
                nc.tensor.matmul(
                    out=ps_o[:], lhsT=ones_t[:], rhs=b_t[:], start=False, stop=True
                )
                o_t = o_pool.tile([P, D], f32, tag="ot")
                nc.scalar.copy(out=o_t[:], in_=ps_o[:])
                nc.sync.dma_start(out=out[t * P : (t + 1) * P, :], in_=o_t[:])

    nc.finalize()
    return nc


def get_program(maxA, maxB):
    key = (tuple(maxA), tuple(maxB))
    if key not in _PROGRAM_CACHE:
        _PROGRAM_CACHE[key] = _build_program(*key)
    return _PROGRAM_CACHE[key]


def kernel(x, src, dst, W, b):
    from concourse.bass_utils import run_bass_kernel_spmd

    in_maps, caps, slot_node = _prepare(x, src, dst, W, b)
    nc = get_program(*caps)
    res = run_bass_kernel_spmd(nc, in_maps, list(range(N_CORES)))

    full = np.empty((N_NODES, D), dtype=np.float32)
    for c in range(N_CORES):
        o = res.results[c]["out"]
        sn = slot_node[c * SLOTS_PER_CORE : (c + 1) * SLOTS_PER_CORE]
        valid = sn >= 0
        full[sn[valid]] = o[valid]
    return full
